# revision 33
# baseline (speedup 1.0000x reference)
"""Trainium2 Bass kernel for nn_GAT_91139206021463.

Two-pass GAT-style multihead attention + FFN, B=32, S=1024, D=768, H=12.
Sharding: data-parallel over batch B across 8 cores (4 batches/core).

v2 structure (vs v1):
  - gce is host-converted to bf16 and loaded feature-major via DMA xbar
    transpose (dma_start_transpose), eliminating the PE transpose matmuls
    and their PSUM->SBUF evacuation copies.
  - Layernorm stats are produced in partition-broadcast form ([128, S])
    directly by using a ones[128,128] stationary operand, so no separate
    row->tile broadcast step is needed.
  - Softmax is left unnormalized; 1/sum enters via per-feature-chunk
    reciprocal columns fused into the attention-apply scalar_tensor_tensor
    ops (which also read the attention-broadcast PSUM directly, skipping
    the bf16 evacuation copies of v1).
  - The topic-path weighted sum uses scalar_tensor_tensor accum_out,
    fusing multiply and free-dim reduction in one DVE op.
  - fcg PSUM->SBUF copies ride on the idle GPSIMD(Pool) engine; FFN relu
    evacuations rotate across ACT/DVE/Pool; output stores issue from the
    Pool software DGE so they never block the SP load queue.

All matmul operands are bf16 (fp32 PSUM accumulation).
"""

import os
import sys
from contextlib import ExitStack

import numpy as np

for _p in ("/opt/trn_rl_repo", "/root/.axon_site/_ro/trn_rl_repo"):
    if os.path.isdir(_p) and _p not in sys.path:
        sys.path.insert(0, _p)

import ml_dtypes  # noqa: E402

import concourse.bass as bass  # noqa: E402
import concourse.tile as tile  # noqa: E402
from concourse import mybir  # noqa: E402
from concourse.bass_utils import run_bass_kernel_spmd  # noqa: E402

B, S, D, H, DH = 32, 1024, 768, 12, 64
DFF = 3 * D
KD = D // 128          # 6 feature chunks
KF = DFF // 128        # 18 ffn chunks
NCORES = 8
NB = B // NCORES       # 4 batches per core
NEG = -1e9

F32 = mybir.dt.float32
BF16 = mybir.dt.bfloat16
BF = ml_dtypes.bfloat16

AX = mybir.AxisListType
AF = mybir.ActivationFunctionType
OP = mybir.AluOpType

# tuning flags
STORES_ON_POOL = os.environ.get("K_ST_POOL", "1") == "1"


# ---------------------------------------------------------------------------
# device program
# ---------------------------------------------------------------------------

def _split_multi_waits(nc, dummy, keep=1):
    """Walrus codegen supports one sync-wait slot per instruction; Tile can
    emit several. Hoist extras onto same-engine EventSemaphore prefixes."""
    upd = mybir.SyncUpdate(sync_type="semaphore", id=dummy.num,
                           ant_name=dummy.name, update_mode="sem-inc",
                           update_value=1)
    ctr = 0
    for fn in nc.m.functions:
        for blk in fn.blocks:
            insts = blk.instructions
            # drop the epilogue EVENT_SEMAPHORE_RANGE_CLEAR: this walrus
            # rejects its encoding ("ISA wrong length"), and sems are
            # zero-initialized at NEFF load (we execute once per load).
            insts[:] = [x for x in insts
                        if getattr(x, "op_name", None)
                        != "EVENT_SEMAPHORE_RANGE_CLEAR"]
            i = 0
            while i < len(insts):
                inst = insts[i]
                si = getattr(inst, "sync_info", None)
                if si is not None and len(si.on_wait) > keep:
                    waits = list(si.on_wait)
                    extra, kept = waits[:-keep], waits[-keep:]
                    for w in extra:
                        ev = mybir.InstEventSemaphore(
                            name=f"wsplit_{ctr}", engine=inst.engine,
                            ins=[], outs=[],
                            sync_info=mybir.SyncInfo(on_wait=[w],
                                                     on_update=[upd]))
                        insts.insert(i, ev)
                        ctr += 1
                        i += 1
                    inst.sync_info = mybir.SyncInfo(
                        on_wait=kept, on_update=list(si.on_update))
                i += 1
    return ctr


def build_program(nb=NB):
    nc = bass.Bass("TRN2", target_bir_lowering=False, debug=False)

    # --- per-core data ---
    gce_d = nc.dram_tensor("gce", [nb, S, D], BF16, kind="ExternalInput").ap()
    negmask_d = nc.dram_tensor("negmask", [nb, S], BF16, kind="ExternalInput").ap()
    topict_d = nc.dram_tensor("topict", [D, nb], BF16, kind="ExternalInput").ap()
    # --- shared weights/constants ---
    wc_d = nc.dram_tensor("wc", [D, D], BF16, kind="ExternalInput").ap()
    wz_d = nc.dram_tensor("wz", [D, 24], BF16, kind="ExternalInput").ap()
    wzt_d = nc.dram_tensor("wzt", [D, 24], BF16, kind="ExternalInput").ap()
    w1_d = nc.dram_tensor("w1", [D, DFF], BF16, kind="ExternalInput").ap()
    w2_d = nc.dram_tensor("w2", [DFF, D], BF16, kind="ExternalInput").ap()
    ea_d = nc.dram_tensor("ea", [24, D], BF16, kind="ExternalInput").ap()
    ea1_d = nc.dram_tensor("ea1", [24, D], BF16, kind="ExternalInput").ap()
    i128b_d = nc.dram_tensor("i128b", [128, 128], BF16, kind="ExternalInput").ap()
    ones128_d = nc.dram_tensor("ones128", [128, 128], BF16, kind="ExternalInput").ap()
    onescol_d = nc.dram_tensor("onescol", [128, 1], BF16, kind="ExternalInput").ap()
    onesrow_d = nc.dram_tensor("onesrow", [1, 128], F32, kind="ExternalInput").ap()
    ones24_d = nc.dram_tensor("ones24", [1, 24], BF16, kind="ExternalInput").ap()
    czt_d = nc.dram_tensor("czt", [24, 1], F32, kind="ExternalInput").ap()

    outgl_d = nc.dram_tensor("outgl", [nb, S, D], F32, kind="ExternalOutput").ap()
    outtp_d = nc.dram_tensor("outtp", [nb, D], F32, kind="ExternalOutput").ap()

    dummy_sem = nc.alloc_semaphore("wsplit_dummy")
    with tile.TileContext(nc) as tc, ExitStack() as ctx:
        wp = ctx.enter_context(tc.tile_pool(name="weights", bufs=1))
        acts = ctx.enter_context(tc.tile_pool(name="acts", bufs=1))
        sm = ctx.enter_context(tc.tile_pool(name="smalls", bufs=2))
        outp = ctx.enter_context(tc.tile_pool(name="outs", bufs=2))
        psb = ctx.enter_context(tc.tile_pool(name="psb", bufs=8, space="PSUM"))

        # ------ small weights first (big FFN weights deferred) ------
        wz_sb = wp.tile([128, KD, 24], BF16)
        nc.sync.dma_start(out=wz_sb, in_=wz_d.rearrange("(k p) d -> p k d", p=128))
        wzt_sb = wp.tile([128, KD, 24], BF16)
        nc.sync.dma_start(out=wzt_sb, in_=wzt_d.rearrange("(k p) d -> p k d", p=128))
        tt_sb = wp.tile([128, KD, nb], BF16)
        nc.sync.dma_start(out=tt_sb, in_=topict_d.rearrange("(k p) b -> p k b", p=128))
        ea_sb = wp.tile([24, D], BF16)
        nc.sync.dma_start(out=ea_sb, in_=ea_d)
        ea1_sb = wp.tile([24, D], BF16)
        nc.sync.dma_start(out=ea1_sb, in_=ea1_d)
        ones128_sb = wp.tile([128, 128], BF16)
        nc.sync.dma_start(out=ones128_sb, in_=ones128_d)
        onescol_sb = wp.tile([128, 1], BF16)
        nc.sync.dma_start(out=onescol_sb, in_=onescol_d)
        onesrow_sb = wp.tile([1, 128], F32)
        nc.sync.dma_start(out=onesrow_sb, in_=onesrow_d)
        ones24_sb = wp.tile([1, 24], BF16)
        nc.sync.dma_start(out=ones24_sb, in_=ones24_d)
        czt_sb = wp.tile([24, 1], F32)
        nc.sync.dma_start(out=czt_sb, in_=czt_d)
        wc_sb = wp.tile([128, KD, D], BF16)
        w1_sb = wp.tile([128, KD, DFF], BF16)
        w2_sb = wp.tile([128, KF, D], BF16)
        i128b_sb = wp.tile([128, 128], BF16)

        eps_sb = {}
        for eps in (1e-5, 1e-6):
            e_t = wp.tile([128, 1], F32, name=f"eps_{eps}")
            nc.vector.memset(e_t, eps)
            eps_sb[eps] = e_t

        # ------ zsrc for pass 1: [24, nb] = wzt.T @ topicT ------
        zs_ps = psb.tile([128, 512], F32, tag="b1")
        for k in range(KD):
            nc.tensor.matmul(zs_ps[0:24, 0:nb], lhsT=wzt_sb[:, k, :],
                             rhs=tt_sb[:, k, :],
                             start=(k == 0), stop=(k == KD - 1))
        zsrc1_sb = wp.tile([24, nb], F32)
        nc.vector.tensor_copy(zsrc1_sb, zs_ps[0:24, 0:nb])

        t2_f = wp.tile([128, KD, nb], F32)    # raw t2 (tanh), fp32
        t2_b = wp.tile([128, KD, nb], BF16)   # raw t2, bf16 (residual lhsT)
        t2ln_sb = wp.tile([128, KD, nb], BF16)

        _namectr = [0]

        def chunk_tiles(tag, bufs, dtype=BF16, w=S):
            out = []
            for k in range(KD):
                _namectr[0] += 1
                out.append(acts.tile([128, w], dtype, tag=tag, bufs=bufs,
                                     name=f"{tag}_{_namectr[0]}_{k}"))
            return out

        # -------- helpers (activations are lists of KD [128, S] tiles) -----

        def feat_ln(src, eps, mb, rb):
            """Partition-direction layernorm stats -> broadcast-form mean /
            rsqrt tiles [128, S] bf16. All sq muls are emitted before the s2
            matmul pass so PE never ping-pongs with DVE per chunk."""
            sq_t = []
            for k in range(KD):
                sq = sm.tile([128, S], BF16, tag="sq", bufs=KD)
                nc.vector.tensor_mul(sq, src[k], src[k])
                sq_t.append(sq)
            for hh in range(2):
                sl = slice(hh * 512, (hh + 1) * 512)
                s1 = psb.tile([128, 512], F32, tag="b1")
                s2 = psb.tile([128, 512], F32, tag="b1")
                for k in range(KD):
                    nc.tensor.matmul(s1, lhsT=ones128_sb, rhs=src[k][:, sl],
                                     start=(k == 0), stop=(k == KD - 1))
                for k in range(KD):
                    nc.tensor.matmul(s2, lhsT=ones128_sb, rhs=sq_t[k][:, sl],
                                     start=(k == 0), stop=(k == KD - 1))
                nc.vector.tensor_scalar(out=mb[:, sl], in0=s1, scalar1=1.0 / D,
                                        scalar2=None, op0=OP.mult)
                msq = sm.tile([128, 512], BF16, tag="msq", bufs=1)
                nc.vector.tensor_mul(msq, mb[:, sl], mb[:, sl])
                var = sm.tile([128, 512], F32, tag="var", bufs=2)
                nc.vector.scalar_tensor_tensor(out=var, in0=s2, scalar=1.0 / D,
                                               in1=msq, op0=OP.mult,
                                               op1=OP.subtract)
                lnv = sm.tile([128, 512], F32, tag="var", bufs=2)
                nc.scalar.activation(lnv, var, AF.Ln, bias=eps_sb[eps])
                nc.scalar.activation(rb[:, sl], lnv, AF.Exp, scale=-0.5)

        def ln_apply(src, mb, rb, dst):
            """dst[k] = src[k]*rb - mb*rb, emitted column-half-major so the
            first consumer (which reads one half of every chunk) can start
            after half the ops."""
            mrb = sm.tile([128, S], BF16, tag="mrb", bufs=1)
            nc.vector.tensor_mul(mrb, mb, rb)
            for hh in range(2):
                sl = slice(hh * 512, (hh + 1) * 512)
                for k in range(KD):
                    tmp = sm.tile([128, 512], BF16, tag="lnt", bufs=3)
                    nc.vector.tensor_mul(tmp, src[k][:, sl], rb[:, sl])
                    nc.vector.tensor_sub(dst[k][:, sl], tmp, mrb[:, sl])

        def vec_ln(tcols, eps):
            """Layernorm of a feature-major vector held as [128, KD] f32 cols."""
            tsq = sm.tile([128, KD], BF16, tag="tsq")
            nc.vector.tensor_mul(tsq, tcols, tcols)
            tcb = sm.tile([128, KD], BF16, tag="tcb")
            nc.vector.tensor_copy(tcb, tcols)
            pm = psb.tile([128, 512], F32, tag="b1")
            pq = psb.tile([128, 512], F32, tag="b1")
            for k in range(KD):
                nc.tensor.matmul(pm[0:1, 0:1], lhsT=onescol_sb,
                                 rhs=tcb[:, k:k + 1],
                                 start=(k == 0), stop=(k == KD - 1))
                nc.tensor.matmul(pq[0:1, 0:1], lhsT=onescol_sb,
                                 rhs=tsq[:, k:k + 1],
                                 start=(k == 0), stop=(k == KD - 1))
            mean = sm.tile([1, 1], F32, tag="tst")
            nc.vector.tensor_scalar(out=mean, in0=pm[0:1, 0:1], scalar1=1.0 / D,
                                    scalar2=None, op0=OP.mult)
            msq = sm.tile([1, 1], F32, tag="tst")
            nc.vector.tensor_mul(msq, mean, mean)
            var = sm.tile([1, 1], F32, tag="tst")
            nc.vector.scalar_tensor_tensor(out=var, in0=pq[0:1, 0:1],
                                           scalar=1.0 / D, in1=msq,
                                           op0=OP.mult, op1=OP.subtract)
            lnv = sm.tile([1, 1], F32, tag="tst")
            nc.scalar.activation(lnv, var, AF.Ln, bias=eps_sb[eps][:1, :])
            rs = sm.tile([1, 1], F32, tag="tst")
            nc.scalar.activation(rs, lnv, AF.Exp, scale=-0.5)
            pbc = psb.tile([128, 512], F32, tag="b1")
            nc.tensor.matmul(pbc[:, 0:1], lhsT=onesrow_sb, rhs=mean,
                             start=True, stop=False)
            nc.tensor.matmul(pbc[:, 1:2], lhsT=onesrow_sb, rhs=rs,
                             start=False, stop=True)
            cols = sm.tile([128, 2], F32, tag="tcols2")
            nc.vector.tensor_copy(cols, pbc[:, 0:2])
            out = sm.tile([128, KD], BF16, tag="tln")
            nc.vector.tensor_scalar(out=out, in0=tcols, scalar1=cols[:, 0:1],
                                    scalar2=cols[:, 1:2], op0=OP.subtract,
                                    op1=OP.mult)
            return out

        def gat_pass(inT, zsrc_col, nm24, gceT, tcols):
            """One multihead pass over chunk lists inT -> gceT; tcols
            [128,KD] f32 pre-tanh weighted sums."""
            zl = sm.tile([24, S], BF16, tag="zl", bufs=1)
            for hh in range(2):
                sl = slice(hh * 512, (hh + 1) * 512)
                zp = psb.tile([128, 512], F32, tag="b1")
                for k in range(KD):
                    nc.tensor.matmul(zp[0:24, :], lhsT=wz_sb[:, k, :],
                                     rhs=inT[k][:, sl],
                                     start=(k == 0), stop=(k == KD - 1))
                zb = sm.tile([24, 512], BF16, tag="zbh", bufs=1)
                nc.vector.scalar_tensor_tensor(out=zb, in0=zp[0:24, :],
                                               scalar=zsrc_col,
                                               in1=nm24[:, sl],
                                               op0=OP.add, op1=OP.add)
                nc.vector.scalar_tensor_tensor(out=zl[:, sl], in0=zb,
                                               scalar=0.01, in1=zb,
                                               op0=OP.mult, op1=OP.max)
            # logits are bounded (~|z| <= 8; masked -> -1e7 -> exp 0), so the
            # usual max-subtraction is unnecessary.
            esum = sm.tile([24, 1], F32, tag="esum")
            a_raw = sm.tile([24, S], BF16, tag="araw", bufs=1)
            nc.scalar.activation(a_raw, zl, AF.Exp, accum_out=esum)
            recip = sm.tile([24, 1], F32, tag="recip")
            nc.vector.reciprocal(recip, esum)
            a_bf = sm.tile([24, S], BF16, tag="abf", bufs=2)
            nc.vector.tensor_scalar(out=a_bf, in0=a_raw, scalar1=recip,
                                    scalar2=None, op0=OP.mult)

            # fcg matmuls run ahead; attention-apply trails by SKEW chunks so
            # the softmax chain never blocks the PE queue head.
            tch = sm.tile([128, 2 * KD], F32, tag="tch")
            fs_t = {}

            def apply_dt(dt):
                dsl = slice(dt * 128, (dt + 1) * 128)
                for hh in range(2):
                    sl = slice(hh * 512, (hh + 1) * 512)
                    pa = psb.tile([128, 512], F32, tag="b1")
                    nc.tensor.matmul(pa, lhsT=ea_sb[:, dsl], rhs=a_bf[:, sl],
                                     start=True, stop=True)
                    pa1 = psb.tile([128, 512], F32, tag="b1")
                    nc.tensor.matmul(pa1, lhsT=ea1_sb[:, dsl], rhs=a_bf[:, sl],
                                     start=True, stop=True)
                    prod = sm.tile([128, 512], BF16, tag="prod", bufs=3)
                    nc.vector.tensor_mul(prod, pa, fs_t[dt][:, sl])
                    nc.scalar.activation(gceT[dt][:, sl], prod, AF.Tanh)
                    junk = sm.tile([128, 512], BF16, tag="prod", bufs=3)
                    nc.vector.scalar_tensor_tensor(
                        out=junk, in0=pa1, scalar=1.0,
                        in1=fs_t[dt][:, sl], op0=OP.mult, op1=OP.mult,
                        accum_out=tch[:, dt * 2 + hh:dt * 2 + hh + 1])

            SKEW = 2
            for dt in range(KD):
                dsl = slice(dt * 128, (dt + 1) * 128)
                fs = sm.tile([128, S], BF16, tag="fs", bufs=SKEW + 1)
                fs_t[dt] = fs
                for hh in range(2):
                    sl = slice(hh * 512, (hh + 1) * 512)
                    fp = psb.tile([128, 512], F32, tag="b1")
                    for k in range(KD):
                        nc.tensor.matmul(fp, lhsT=wc_sb[:, k, dsl],
                                         rhs=inT[k][:, sl],
                                         start=(k == 0), stop=(k == KD - 1))
                    nc.scalar.activation(fs[:, sl], fp, AF.Copy)
                if dt >= SKEW:
                    apply_dt(dt - SKEW)
            for dt in range(KD - SKEW, KD):
                apply_dt(dt)
            tchv = tch.rearrange("p (d h) -> p d h", h=2)
            nc.vector.tensor_add(tcols, tchv[:, :, 0], tchv[:, :, 1])

        # ================= software-pipelined batch loop =================
        st = [dict() for _ in range(nb)]

        def stage_T(b):
            ngm = sm.tile([1, S], BF16, tag="ngm", bufs=2)
            nc.sync.dma_start(out=ngm, in_=negmask_d[b:b + 1, :])
            g0t = chunk_tiles("actA", 12)
            for dt in range(KD):
                nc.sync.dma_start_transpose(
                    out=g0t[dt],
                    in_=gce_d[b, :, dt * 128:(dt + 1) * 128])
            nm24 = sm.tile([24, S], BF16, tag="nm24", bufs=2)
            for hh in range(2):
                sl = slice(hh * 512, (hh + 1) * 512)
                nmp = psb.tile([128, 512], F32, tag="b1")
                nc.tensor.matmul(nmp[0:24, :], lhsT=ones24_sb,
                                 rhs=ngm[:, sl], start=True, stop=True)
                nc.vector.tensor_copy(nm24[:, sl], nmp[0:24, :])
            st[b]["g0t"], st[b]["nm24"] = g0t, nm24

        def stage_P1(b):
            gce1 = chunk_tiles("actB", 12)
            t1c = sm.tile([128, KD], F32, tag="t1c", bufs=2)
            gat_pass(st[b]["g0t"], zsrc1_sb[:, b:b + 1], st[b]["nm24"],
                     gce1, t1c)
            st[b]["gce1"], st[b]["t1c"] = gce1, t1c

        def stage_V1(b):
            """t1 layernorm folded into zsrc2 algebraically:
            zsrc2 = r*(wzt.T@t1c - m*colsum(wzt)), so the only PE matmuls
            wait just on the t1c tanh, not on the whole stats chain."""
            t1c = st[b]["t1c"]
            nc.scalar.activation(t1c, t1c, AF.Tanh)
            tsq = sm.tile([128, KD], BF16, tag="tsq")
            nc.vector.tensor_mul(tsq, t1c, t1c)
            tcb = sm.tile([128, KD], BF16, tag="tcb")
            nc.vector.tensor_copy(tcb, t1c)
            pm = psb.tile([128, 512], F32, tag="b1")
            pq = psb.tile([128, 512], F32, tag="b1")
            zraw = psb.tile([128, 512], F32, tag="b1")
            for k in range(KD):
                nc.tensor.matmul(pm[0:1, 0:1], lhsT=onescol_sb,
                                 rhs=tcb[:, k:k + 1],
                                 start=(k == 0), stop=(k == KD - 1))
                nc.tensor.matmul(pq[0:1, 0:1], lhsT=onescol_sb,
                                 rhs=tsq[:, k:k + 1],
                                 start=(k == 0), stop=(k == KD - 1))
                nc.tensor.matmul(zraw[0:24, 0:1], lhsT=wzt_sb[:, k, :],
                                 rhs=tcb[:, k:k + 1],
                                 start=(k == 0), stop=(k == KD - 1))
            mean = sm.tile([1, 1], F32, tag="tst")
            nc.vector.tensor_scalar(out=mean, in0=pm[0:1, 0:1], scalar1=1.0 / D,
                                    scalar2=None, op0=OP.mult)
            msq = sm.tile([1, 1], F32, tag="tst")
            nc.vector.tensor_mul(msq, mean, mean)
            var = sm.tile([1, 1], F32, tag="tst")
            nc.vector.scalar_tensor_tensor(out=var, in0=pq[0:1, 0:1],
                                           scalar=1.0 / D, in1=msq,
                                           op0=OP.mult, op1=OP.subtract)
            lnv = sm.tile([1, 1], F32, tag="tst")
            nc.scalar.activation(lnv, var, AF.Ln, bias=eps_sb[1e-5][:1, :])
            rs = sm.tile([1, 1], F32, tag="tst")
            nc.scalar.activation(rs, lnv, AF.Exp, scale=-0.5)
            # broadcast mean/rs to 24 partitions (free-size-1 matmuls)
            p24 = psb.tile([128, 512], F32, tag="b1")
            nc.tensor.matmul(p24[0:24, 0:1], lhsT=onesrow_sb[:, 0:24],
                             rhs=mean, start=True, stop=False)
            nc.tensor.matmul(p24[0:24, 1:2], lhsT=onesrow_sb[:, 0:24],
                             rhs=rs, start=False, stop=True)
            mr24 = sm.tile([24, 2], F32, tag="mr24", bufs=2)
            nc.vector.tensor_copy(mr24, p24[0:24, 0:2])
            mc24 = sm.tile([24, 1], F32, tag="mr24", bufs=2)
            nc.vector.tensor_mul(mc24, mr24[:, 0:1], czt_sb)
            zsrc2 = sm.tile([24, 1], F32, tag="zsrc2", bufs=2)
            nc.vector.scalar_tensor_tensor(out=zsrc2, in0=zraw[0:24, 0:1],
                                           scalar=mc24, in1=mr24[:, 1:2],
                                           op0=OP.subtract, op1=OP.mult)
            st[b]["zsrc2"] = zsrc2

        def stage_S1(b):
            mb1 = sm.tile([128, S], BF16, tag="lnmb", bufs=2)
            rb1 = sm.tile([128, S], BF16, tag="lnrb", bufs=2)
            feat_ln(st[b]["gce1"], 1e-5, mb1, rb1)
            st[b]["mb1"], st[b]["rb1"] = mb1, rb1

        def stage_A1(b):
            g1ln = chunk_tiles("actB", 12)
            ln_apply(st[b]["gce1"], st[b]["mb1"], st[b]["rb1"], g1ln)
            st[b]["g1ln"] = g1ln

        def stage_P2(b):
            gce2 = chunk_tiles("actA", 12)
            gat_pass(st[b]["g1ln"], st[b]["zsrc2"], st[b]["nm24"],
                     gce2, t2_f[:, :, b])
            st[b]["gce2"] = gce2

        def stage_V2(b):
            # t2 tanh + layernorm for the out_tp tail; pure DVE/ACT latency,
            # emitted while the FFN keeps PE busy.
            nc.scalar.activation(t2_f[:, :, b], t2_f[:, :, b], AF.Tanh)
            nc.vector.tensor_copy(t2_b[:, :, b], t2_f[:, :, b])
            t2ln = vec_ln(t2_f[:, :, b], 1e-6)
            nc.vector.tensor_copy(t2ln_sb[:, :, b], t2ln)

        def stage_S2(b):
            mb2 = sm.tile([128, S], BF16, tag="lnmb", bufs=2)
            rb2 = sm.tile([128, S], BF16, tag="lnrb", bufs=2)
            feat_ln(st[b]["gce2"], 1e-6, mb2, rb2)
            st[b]["mb2"], st[b]["rb2"] = mb2, rb2

        def stage_A2(b):
            g2ln = chunk_tiles("actB", 12)
            ln_apply(st[b]["gce2"], st[b]["mb2"], st[b]["rb2"], g2ln)
            st[b]["g2ln"] = g2ln

        def stage_F(b):
            gce2, g2ln = st[b]["gce2"], st[b]["g2ln"]
            for qq in range(4):
                qsl = slice(qq * 256, (qq + 1) * 256)
                _namectr[0] += 1
                intT = [acts.tile([128, 256], BF16, tag="intT", bufs=36,
                                  name=f"intT_{_namectr[0]}_{f}")
                        for f in range(KF)]
                for f in range(KF):
                    ip = psb.tile([128, 512], F32, tag="b1")
                    for k in range(KD):
                        nc.tensor.matmul(ip[:, 0:256],
                                         lhsT=w1_sb[:, k, f * 128:(f + 1) * 128],
                                         rhs=g2ln[k][:, qsl],
                                         start=(k == 0), stop=(k == KD - 1))
                    if f % 2 == 0:
                        nc.scalar.activation(intT[f], ip[:, 0:256], AF.Relu)
                    else:
                        nc.vector.tensor_scalar_max(intT[f], ip[:, 0:256], 0.0)
                for j4 in range(2):
                    sj = qq * 2 + j4
                    jsl = slice(j4 * 128, (j4 + 1) * 128)
                    for half in range(2):
                        osl = slice(half * 384, (half + 1) * 384)
                        op_ = psb.tile([128, 512], F32, tag="b1")
                        for f in range(KF):
                            nc.tensor.matmul(op_[:, 0:384],
                                             lhsT=intT[f][:, jsl],
                                             rhs=w2_sb[:, f, osl],
                                             start=(f == 0), stop=False)
                        for jj in range(3):
                            j = half * 3 + jj
                            nc.tensor.matmul(op_[:, jj * 128:(jj + 1) * 128],
                                             lhsT=gce2[j][:, sj * 128:(sj + 1) * 128],
                                             rhs=i128b_sb, start=False,
                                             stop=(jj == 2))
                        osb = outp.tile([128, 384], F32, tag="osb")
                        nc.scalar.activation(osb, op_[:, 0:384], AF.Copy)
                        if STORES_ON_POOL:
                            nc.gpsimd.dma_start(
                                out=outgl_d[b, sj * 128:(sj + 1) * 128, osl],
                                in_=osb)
                        else:
                            nc.sync.dma_start(
                                out=outgl_d[b, sj * 128:(sj + 1) * 128, osl],
                                in_=osb)

        stage_T(0)
        # big FFN weights load behind the first transposes
        nc.sync.dma_start(out=wc_sb, in_=wc_d.rearrange("(k p) d -> p k d", p=128))
        nc.sync.dma_start(out=w1_sb, in_=w1_d.rearrange("(k p) d -> p k d", p=128))
        nc.sync.dma_start(out=w2_sb, in_=w2_d.rearrange("(k p) d -> p k d", p=128))
        nc.sync.dma_start(out=i128b_sb, in_=i128b_d)
        stage_P1(0)
        stage_S1(0)
        stage_V1(0)
        for b in range(nb):
            stage_A1(b)
            stage_P2(b)
            stage_S2(b)
            if b + 1 < nb:
                stage_T(b + 1)
                stage_P1(b + 1)
                stage_S1(b + 1)
            stage_A2(b)
            stage_F(b)
            if b + 1 < nb:
                stage_V1(b + 1)
            stage_V2(b)

        # ================= out_tp rows (all batches) =================
        itp = psb.tile([128, 512], F32, tag="b1")
        for f in range(KF):
            for k in range(KD):
                nc.tensor.matmul(itp[:, f * nb:(f + 1) * nb],
                                 lhsT=w1_sb[:, k, f * 128:(f + 1) * 128],
                                 rhs=t2ln_sb[:, k, :],
                                 start=(f == 0 and k == 0),
                                 stop=(f == KF - 1 and k == KD - 1))
        itp_sb = sm.tile([128, KF * nb], BF16, tag="itp")
        nc.scalar.activation(itp_sb, itp[:, 0:KF * nb], AF.Relu)
        for half in range(2):
            osl = slice(half * 384, (half + 1) * 384)
            otp = psb.tile([128, 512], F32, tag="b1")
            for f in range(KF):
                nc.tensor.matmul(otp[0:nb, 0:384],
                                 lhsT=itp_sb[:, f * nb:(f + 1) * nb],
                                 rhs=w2_sb[:, f, osl],
                                 start=(f == 0), stop=False)
            for jj in range(3):
                j = half * 3 + jj
                nc.tensor.matmul(otp[0:nb, jj * 128:(jj + 1) * 128],
                                 lhsT=t2_b[:, j, :],
                                 rhs=i128b_sb, start=False, stop=(jj == 2))
            otp_sb = outp.tile([nb, 384], F32, tag="otp")
            nc.scalar.activation(otp_sb, otp[0:nb, 0:384], AF.Copy)
            nc.sync.dma_start(out=outtp_d[:, osl], in_=otp_sb)

    _split_multi_waits(nc, dummy_sem)
    return nc


# ---------------------------------------------------------------------------
# host side
# ---------------------------------------------------------------------------

def host_prep(inputs):
    """Fold weights; build constants. Returns dict of shared arrays."""
    Wt = np.asarray(inputs["Wt"], np.float32)
    Wg = np.asarray(inputs["Wg"], np.float32)
    Wc = np.asarray(inputs["Wc"], np.float32)
    Wa = np.asarray(inputs["Wa"], np.float32)
    Wa1 = np.asarray(inputs["Wa1"], np.float32)

    wc = np.ascontiguousarray(np.transpose(Wc, (1, 0, 2)).reshape(D, D))
    wz = np.concatenate([np.einsum("hid,hd->ih", Wg, Wa[:, DH:]),
                         np.einsum("hid,hd->ih", Wg, Wa1[:, DH:])], axis=1)
    wzt = np.concatenate([np.einsum("hid,hd->ih", Wt, Wa[:, :DH]),
                          np.einsum("hid,hd->ih", Wt, Wa1[:, :DH])], axis=1)

    hmap = (np.arange(D) // DH)  # feature -> head
    ea = np.zeros((24, D), np.float32)
    ea[hmap, np.arange(D)] = 1.0          # rows 0..11 select attn-a
    ea1 = np.zeros((24, D), np.float32)
    ea1[12 + hmap, np.arange(D)] = 1.0    # rows 12..23 select attn-a1

    return {
        "wc": wc.astype(BF), "wz": wz.astype(BF), "wzt": wzt.astype(BF),
        "w1": np.asarray(inputs["pw_w1"], np.float32).astype(BF),
        "w2": np.asarray(inputs["pw_w2"], np.float32).astype(BF),
        "ea": ea.astype(BF), "ea1": ea1.astype(BF),
        "i128b": np.eye(128, dtype=np.float32).astype(BF),
        "ones128": np.ones((128, 128), np.float32).astype(BF),
        "onescol": np.ones((128, 1), np.float32).astype(BF),
        "onesrow": np.ones((1, 128), np.float32),
        "ones24": np.ones((1, 24), np.float32).astype(BF),
        "czt": np.ascontiguousarray(wzt.sum(axis=0).reshape(24, 1)),
    }


def core_inputs(inputs, shared, c, nb=NB):
    """Per-core in_map (core c takes batches c*nb .. c*nb+nb)."""
    sl = slice(c * nb, c * nb + nb)
    gce = np.ascontiguousarray(np.asarray(
        inputs["global_context_embed"], np.float32)[sl]).astype(BF)
    mask = np.asarray(inputs["mask"])[sl]
    negmask = np.where(mask, np.float32(NEG), np.float32(0.0)).astype(BF)
    topict = np.ascontiguousarray(
        np.asarray(inputs["topic_embed"], np.float32).T[:, sl]).astype(BF)
    m = dict(shared)
    m.update({"gce": gce, "negmask": negmask, "topict": topict})
    return m


_prog_cache = {}


def _get_program(nb=NB):
    if nb not in _prog_cache:
        _prog_cache[nb] = build_program(nb)
    return _prog_cache[nb]


def kernel(**inputs):
    nc = _get_program()
    shared = host_prep(inputs)
    in_maps = [core_inputs(inputs, shared, c) for c in range(NCORES)]
    res = run_bass_kernel_spmd(nc, in_maps, list(range(NCORES)))
    outgl = np.concatenate([res.results[c]["outgl"] for c in range(NCORES)], axis=0)
    tprow = np.concatenate([res.results[c]["outtp"] for c in range(NCORES)], axis=0)
    out_tp = np.broadcast_to(tprow[:, None, :], (B, S, D))
    return np.ascontiguousarray(outgl), np.ascontiguousarray(out_tp)


# revision 36
# speedup vs baseline: 1.0285x; 1.0285x over previous
"""Trainium2 Bass kernel for nn_GAT_91139206021463.

Two-pass GAT-style multihead attention + FFN, B=32, S=1024, D=768, H=12.
Sharding: data-parallel over batch B across 8 cores (4 batches/core).

v2 structure (vs v1):
  - gce is host-converted to bf16 and loaded feature-major via DMA xbar
    transpose (dma_start_transpose), eliminating the PE transpose matmuls
    and their PSUM->SBUF evacuation copies.
  - Layernorm stats are produced in partition-broadcast form ([128, S])
    directly by using a ones[128,128] stationary operand, so no separate
    row->tile broadcast step is needed.
  - Softmax is left unnormalized; 1/sum enters via per-feature-chunk
    reciprocal columns fused into the attention-apply scalar_tensor_tensor
    ops (which also read the attention-broadcast PSUM directly, skipping
    the bf16 evacuation copies of v1).
  - The topic-path weighted sum uses scalar_tensor_tensor accum_out,
    fusing multiply and free-dim reduction in one DVE op.
  - fcg PSUM->SBUF copies ride on the idle GPSIMD(Pool) engine; FFN relu
    evacuations rotate across ACT/DVE/Pool; output stores issue from the
    Pool software DGE so they never block the SP load queue.

All matmul operands are bf16 (fp32 PSUM accumulation).
"""

import os
import sys
from contextlib import ExitStack

import numpy as np

for _p in ("/opt/trn_rl_repo", "/root/.axon_site/_ro/trn_rl_repo"):
    if os.path.isdir(_p) and _p not in sys.path:
        sys.path.insert(0, _p)

import ml_dtypes  # noqa: E402

import concourse.bass as bass  # noqa: E402
import concourse.tile as tile  # noqa: E402
from concourse import mybir  # noqa: E402
from concourse.bass_utils import run_bass_kernel_spmd  # noqa: E402

B, S, D, H, DH = 32, 1024, 768, 12, 64
DFF = 3 * D
KD = D // 128          # 6 feature chunks
KF = DFF // 128        # 18 ffn chunks
NCORES = 8
NB = B // NCORES       # 4 batches per core
NEG = -1e9

F32 = mybir.dt.float32
BF16 = mybir.dt.bfloat16
BF = ml_dtypes.bfloat16

AX = mybir.AxisListType
AF = mybir.ActivationFunctionType
OP = mybir.AluOpType

# tuning flags
STORES_ON_POOL = os.environ.get("K_ST_POOL", "1") == "1"


# ---------------------------------------------------------------------------
# device program
# ---------------------------------------------------------------------------

def _split_multi_waits(nc, dummy, keep=1):
    """Walrus codegen supports one sync-wait slot per instruction; Tile can
    emit several. Hoist extras onto same-engine EventSemaphore prefixes."""
    upd = mybir.SyncUpdate(sync_type="semaphore", id=dummy.num,
                           ant_name=dummy.name, update_mode="sem-inc",
                           update_value=1)
    ctr = 0
    for fn in nc.m.functions:
        for blk in fn.blocks:
            insts = blk.instructions
            # drop the epilogue EVENT_SEMAPHORE_RANGE_CLEAR: this walrus
            # rejects its encoding ("ISA wrong length"), and sems are
            # zero-initialized at NEFF load (we execute once per load).
            insts[:] = [x for x in insts
                        if getattr(x, "op_name", None)
                        != "EVENT_SEMAPHORE_RANGE_CLEAR"]
            i = 0
            while i < len(insts):
                inst = insts[i]
                si = getattr(inst, "sync_info", None)
                if si is not None and len(si.on_wait) > keep:
                    waits = list(si.on_wait)
                    extra, kept = waits[:-keep], waits[-keep:]
                    for w in extra:
                        ev = mybir.InstEventSemaphore(
                            name=f"wsplit_{ctr}", engine=inst.engine,
                            ins=[], outs=[],
                            sync_info=mybir.SyncInfo(on_wait=[w],
                                                     on_update=[upd]))
                        insts.insert(i, ev)
                        ctr += 1
                        i += 1
                    inst.sync_info = mybir.SyncInfo(
                        on_wait=kept, on_update=list(si.on_update))
                i += 1
    return ctr


def build_program(nb=NB):
    nc = bass.Bass("TRN2", target_bir_lowering=False, debug=False)

    # --- per-core data ---
    gce_d = nc.dram_tensor("gce", [nb, S, D], BF16, kind="ExternalInput").ap()
    negmask_d = nc.dram_tensor("negmask", [nb, S], BF16, kind="ExternalInput").ap()
    topict_d = nc.dram_tensor("topict", [D, nb], BF16, kind="ExternalInput").ap()
    # --- shared weights/constants ---
    wc_d = nc.dram_tensor("wc", [D, D], BF16, kind="ExternalInput").ap()
    wz_d = nc.dram_tensor("wz", [D, 24], BF16, kind="ExternalInput").ap()
    wzt_d = nc.dram_tensor("wzt", [D, 24], BF16, kind="ExternalInput").ap()
    w1_d = nc.dram_tensor("w1", [D, DFF], BF16, kind="ExternalInput").ap()
    w2_d = nc.dram_tensor("w2", [DFF, D], BF16, kind="ExternalInput").ap()
    ea_d = nc.dram_tensor("ea", [24, D], BF16, kind="ExternalInput").ap()
    ea1_d = nc.dram_tensor("ea1", [24, D], BF16, kind="ExternalInput").ap()
    i128b_d = nc.dram_tensor("i128b", [128, 128], BF16, kind="ExternalInput").ap()
    ones128_d = nc.dram_tensor("ones128", [128, 128], BF16, kind="ExternalInput").ap()
    onescol_d = nc.dram_tensor("onescol", [128, 1], BF16, kind="ExternalInput").ap()
    onesrow_d = nc.dram_tensor("onesrow", [1, 128], F32, kind="ExternalInput").ap()
    ones24_d = nc.dram_tensor("ones24", [1, 24], BF16, kind="ExternalInput").ap()
    czt_d = nc.dram_tensor("czt", [24, 1], F32, kind="ExternalInput").ap()

    outgl_d = nc.dram_tensor("outgl", [nb, S, D], F32, kind="ExternalOutput").ap()
    outtp_d = nc.dram_tensor("outtp", [nb, D], F32, kind="ExternalOutput").ap()

    dummy_sem = nc.alloc_semaphore("wsplit_dummy")
    with tile.TileContext(nc) as tc, ExitStack() as ctx:
        wp = ctx.enter_context(tc.tile_pool(name="weights", bufs=1))
        acts = ctx.enter_context(tc.tile_pool(name="acts", bufs=1))
        sm = ctx.enter_context(tc.tile_pool(name="smalls", bufs=2))
        outp = ctx.enter_context(tc.tile_pool(name="outs", bufs=2))
        psb = ctx.enter_context(tc.tile_pool(name="psb", bufs=8, space="PSUM"))

        # ------ small weights first (big FFN weights deferred) ------
        wz_sb = wp.tile([128, KD, 24], BF16)
        nc.sync.dma_start(out=wz_sb, in_=wz_d.rearrange("(k p) d -> p k d", p=128))
        wzt_sb = wp.tile([128, KD, 24], BF16)
        nc.sync.dma_start(out=wzt_sb, in_=wzt_d.rearrange("(k p) d -> p k d", p=128))
        tt_sb = wp.tile([128, KD, nb], BF16)
        nc.sync.dma_start(out=tt_sb, in_=topict_d.rearrange("(k p) b -> p k b", p=128))
        ea_sb = wp.tile([24, D], BF16)
        nc.sync.dma_start(out=ea_sb, in_=ea_d)
        ea1_sb = wp.tile([24, D], BF16)
        nc.sync.dma_start(out=ea1_sb, in_=ea1_d)
        ones128_sb = wp.tile([128, 128], BF16)
        nc.sync.dma_start(out=ones128_sb, in_=ones128_d)
        onescol_sb = wp.tile([128, 1], BF16)
        nc.sync.dma_start(out=onescol_sb, in_=onescol_d)
        onesrow_sb = wp.tile([1, 128], F32)
        nc.sync.dma_start(out=onesrow_sb, in_=onesrow_d)
        ones24_sb = wp.tile([1, 24], BF16)
        nc.sync.dma_start(out=ones24_sb, in_=ones24_d)
        czt_sb = wp.tile([24, 1], F32)
        nc.sync.dma_start(out=czt_sb, in_=czt_d)
        wc_sb = [wp.tile([128, D], BF16, name=f"wc_sb{k}") for k in range(KD)]
        w1_sb = wp.tile([128, KD, DFF], BF16)
        w2_sb = wp.tile([128, KF, D], BF16)
        i128b_sb = wp.tile([128, 128], BF16)

        eps_sb = {}
        for eps in (1e-5, 1e-6):
            e_t = wp.tile([128, 1], F32, name=f"eps_{eps}")
            nc.vector.memset(e_t, eps)
            eps_sb[eps] = e_t

        # ------ zsrc for pass 1: [24, nb] = wzt.T @ topicT ------
        zs_ps = psb.tile([128, 512], F32, tag="b1")
        for k in range(KD):
            nc.tensor.matmul(zs_ps[0:24, 0:nb], lhsT=wzt_sb[:, k, :],
                             rhs=tt_sb[:, k, :],
                             start=(k == 0), stop=(k == KD - 1))
        zsrc1_sb = wp.tile([24, nb], F32)
        nc.vector.tensor_copy(zsrc1_sb, zs_ps[0:24, 0:nb])

        t2_f = wp.tile([128, KD, nb], F32)    # raw t2 (tanh), fp32
        t2_b = wp.tile([128, KD, nb], BF16)   # raw t2, bf16 (residual lhsT)
        t2ln_sb = wp.tile([128, KD, nb], BF16)

        _namectr = [0]

        def chunk_tiles(tag, bufs, dtype=BF16, w=S):
            out = []
            for k in range(KD):
                _namectr[0] += 1
                out.append(acts.tile([128, w], dtype, tag=tag, bufs=bufs,
                                     name=f"{tag}_{_namectr[0]}_{k}"))
            return out

        # -------- helpers (activations are lists of KD [128, S] tiles) -----

        def feat_ln(src, eps, mb, rb):
            """Partition-direction layernorm stats -> broadcast-form mean /
            rsqrt tiles [128, S] bf16. All sq muls are emitted before the s2
            matmul pass so PE never ping-pongs with DVE per chunk."""
            sq_t = []
            for k in range(KD):
                sq = sm.tile([128, S], BF16, tag="sq", bufs=KD)
                nc.vector.tensor_mul(sq, src[k], src[k])
                sq_t.append(sq)
            for hh in range(2):
                sl = slice(hh * 512, (hh + 1) * 512)
                s1 = psb.tile([128, 512], F32, tag="b1")
                s2 = psb.tile([128, 512], F32, tag="b1")
                for k in range(KD):
                    nc.tensor.matmul(s1, lhsT=ones128_sb, rhs=src[k][:, sl],
                                     start=(k == 0), stop=(k == KD - 1))
                for k in range(KD):
                    nc.tensor.matmul(s2, lhsT=ones128_sb, rhs=sq_t[k][:, sl],
                                     start=(k == 0), stop=(k == KD - 1))
                nc.vector.tensor_scalar(out=mb[:, sl], in0=s1, scalar1=1.0 / D,
                                        scalar2=None, op0=OP.mult)
                msq = sm.tile([128, 512], BF16, tag="msq", bufs=1)
                nc.vector.tensor_mul(msq, mb[:, sl], mb[:, sl])
                var = sm.tile([128, 512], F32, tag="var", bufs=2)
                nc.vector.scalar_tensor_tensor(out=var, in0=s2, scalar=1.0 / D,
                                               in1=msq, op0=OP.mult,
                                               op1=OP.subtract)
                lnv = sm.tile([128, 512], F32, tag="var", bufs=2)
                nc.scalar.activation(lnv, var, AF.Ln, bias=eps_sb[eps])
                nc.scalar.activation(rb[:, sl], lnv, AF.Exp, scale=-0.5)

        def ln_apply(src, mb, rb, dst):
            """dst[k] = src[k]*rb - mb*rb, emitted column-half-major so the
            first consumer (which reads one half of every chunk) can start
            after half the ops."""
            mrb = sm.tile([128, S], BF16, tag="mrb", bufs=1)
            nc.vector.tensor_mul(mrb, mb, rb)
            for hh in range(2):
                sl = slice(hh * 512, (hh + 1) * 512)
                for k in range(KD):
                    tmp = sm.tile([128, 512], BF16, tag="lnt", bufs=3)
                    nc.vector.tensor_mul(tmp, src[k][:, sl], rb[:, sl])
                    nc.vector.tensor_sub(dst[k][:, sl], tmp, mrb[:, sl])

        def vec_ln(tcols, eps):
            """Layernorm of a feature-major vector held as [128, KD] f32 cols."""
            tsq = sm.tile([128, KD], BF16, tag="tsq")
            nc.vector.tensor_mul(tsq, tcols, tcols)
            tcb = sm.tile([128, KD], BF16, tag="tcb")
            nc.vector.tensor_copy(tcb, tcols)
            pm = psb.tile([128, 512], F32, tag="b1")
            pq = psb.tile([128, 512], F32, tag="b1")
            for k in range(KD):
                nc.tensor.matmul(pm[0:1, 0:1], lhsT=onescol_sb,
                                 rhs=tcb[:, k:k + 1],
                                 start=(k == 0), stop=(k == KD - 1))
                nc.tensor.matmul(pq[0:1, 0:1], lhsT=onescol_sb,
                                 rhs=tsq[:, k:k + 1],
                                 start=(k == 0), stop=(k == KD - 1))
            mean = sm.tile([1, 1], F32, tag="tst")
            nc.vector.tensor_scalar(out=mean, in0=pm[0:1, 0:1], scalar1=1.0 / D,
                                    scalar2=None, op0=OP.mult)
            msq = sm.tile([1, 1], F32, tag="tst")
            nc.vector.tensor_mul(msq, mean, mean)
            var = sm.tile([1, 1], F32, tag="tst")
            nc.vector.scalar_tensor_tensor(out=var, in0=pq[0:1, 0:1],
                                           scalar=1.0 / D, in1=msq,
                                           op0=OP.mult, op1=OP.subtract)
            lnv = sm.tile([1, 1], F32, tag="tst")
            nc.scalar.activation(lnv, var, AF.Ln, bias=eps_sb[eps][:1, :])
            rs = sm.tile([1, 1], F32, tag="tst")
            nc.scalar.activation(rs, lnv, AF.Exp, scale=-0.5)
            pbc = psb.tile([128, 512], F32, tag="b1")
            nc.tensor.matmul(pbc[:, 0:1], lhsT=onesrow_sb, rhs=mean,
                             start=True, stop=False)
            nc.tensor.matmul(pbc[:, 1:2], lhsT=onesrow_sb, rhs=rs,
                             start=False, stop=True)
            cols = sm.tile([128, 2], F32, tag="tcols2")
            nc.vector.tensor_copy(cols, pbc[:, 0:2])
            out = sm.tile([128, KD], BF16, tag="tln")
            nc.vector.tensor_scalar(out=out, in0=tcols, scalar1=cols[:, 0:1],
                                    scalar2=cols[:, 1:2], op0=OP.subtract,
                                    op1=OP.mult)
            return out

        def gat_pass(inT, zsrc_col, nm24, gceT, tcols):
            """One multihead pass over chunk lists inT -> gceT; tcols
            [128,KD] f32 pre-tanh weighted sums."""
            zl = sm.tile([24, S], BF16, tag="zl", bufs=1)
            for hh in range(2):
                sl = slice(hh * 512, (hh + 1) * 512)
                zp = psb.tile([128, 512], F32, tag="b1")
                for k in range(KD):
                    nc.tensor.matmul(zp[0:24, :], lhsT=wz_sb[:, k, :],
                                     rhs=inT[k][:, sl],
                                     start=(k == 0), stop=(k == KD - 1))
                zb = sm.tile([24, 512], BF16, tag="zbh", bufs=1)
                nc.vector.scalar_tensor_tensor(out=zb, in0=zp[0:24, :],
                                               scalar=zsrc_col,
                                               in1=nm24[:, sl],
                                               op0=OP.add, op1=OP.add)
                nc.vector.scalar_tensor_tensor(out=zl[:, sl], in0=zb,
                                               scalar=0.01, in1=zb,
                                               op0=OP.mult, op1=OP.max)
            # logits are bounded (~|z| <= 8; masked -> -1e7 -> exp 0), so the
            # usual max-subtraction is unnecessary.
            esum = sm.tile([24, 1], F32, tag="esum")
            a_raw = sm.tile([24, S], BF16, tag="araw", bufs=1)
            nc.scalar.activation(a_raw, zl, AF.Exp, accum_out=esum)
            recip = sm.tile([24, 1], F32, tag="recip")
            nc.vector.reciprocal(recip, esum)
            a_bf = sm.tile([24, S], BF16, tag="abf", bufs=2)
            nc.vector.tensor_scalar(out=a_bf, in0=a_raw, scalar1=recip,
                                    scalar2=None, op0=OP.mult)

            # fcg matmuls run ahead; attention-apply trails by SKEW chunks so
            # the softmax chain never blocks the PE queue head.
            tch = sm.tile([128, 2 * KD], F32, tag="tch")
            fs_t = {}

            def apply_dt(dt):
                dsl = slice(dt * 128, (dt + 1) * 128)
                for hh in range(2):
                    sl = slice(hh * 512, (hh + 1) * 512)
                    pa = psb.tile([128, 512], F32, tag="b1")
                    nc.tensor.matmul(pa, lhsT=ea_sb[:, dsl], rhs=a_bf[:, sl],
                                     start=True, stop=True)
                    pa1 = psb.tile([128, 512], F32, tag="b1")
                    nc.tensor.matmul(pa1, lhsT=ea1_sb[:, dsl], rhs=a_bf[:, sl],
                                     start=True, stop=True)
                    prod = sm.tile([128, 512], BF16, tag="prod", bufs=3)
                    nc.vector.tensor_mul(prod, pa, fs_t[dt][:, sl])
                    nc.scalar.activation(gceT[dt][:, sl], prod, AF.Tanh)
                    junk = sm.tile([128, 512], BF16, tag="prod", bufs=3)
                    nc.vector.scalar_tensor_tensor(
                        out=junk, in0=pa1, scalar=1.0,
                        in1=fs_t[dt][:, sl], op0=OP.mult, op1=OP.mult,
                        accum_out=tch[:, dt * 2 + hh:dt * 2 + hh + 1])

            SKEW = 2
            for dt in range(KD):
                dsl = slice(dt * 128, (dt + 1) * 128)
                fs = sm.tile([128, S], BF16, tag="fs", bufs=SKEW + 1)
                fs_t[dt] = fs
                for hh in range(2):
                    sl = slice(hh * 512, (hh + 1) * 512)
                    fp = psb.tile([128, 512], F32, tag="b1")
                    for k in range(KD):
                        nc.tensor.matmul(fp, lhsT=wc_sb[k][:, dsl],
                                         rhs=inT[k][:, sl],
                                         start=(k == 0), stop=(k == KD - 1))
                    nc.scalar.activation(fs[:, sl], fp, AF.Copy)
                if dt >= SKEW:
                    apply_dt(dt - SKEW)
            for dt in range(KD - SKEW, KD):
                apply_dt(dt)
            tchv = tch.rearrange("p (d h) -> p d h", h=2)
            nc.vector.tensor_add(tcols, tchv[:, :, 0], tchv[:, :, 1])

        # ================= software-pipelined batch loop =================
        st = [dict() for _ in range(nb)]

        def stage_T(b):
            ngm = sm.tile([1, S], BF16, tag="ngm", bufs=2)
            nc.sync.dma_start(out=ngm, in_=negmask_d[b:b + 1, :])
            g0t = chunk_tiles("actA", 12)
            for dt in range(KD):
                nc.sync.dma_start_transpose(
                    out=g0t[dt],
                    in_=gce_d[b, :, dt * 128:(dt + 1) * 128])
            nm24 = sm.tile([24, S], BF16, tag="nm24", bufs=2)
            for hh in range(2):
                sl = slice(hh * 512, (hh + 1) * 512)
                nmp = psb.tile([128, 512], F32, tag="b1")
                nc.tensor.matmul(nmp[0:24, :], lhsT=ones24_sb,
                                 rhs=ngm[:, sl], start=True, stop=True)
                nc.vector.tensor_copy(nm24[:, sl], nmp[0:24, :])
            st[b]["g0t"], st[b]["nm24"] = g0t, nm24

        def stage_P1(b):
            gce1 = chunk_tiles("actB", 12)
            t1c = sm.tile([128, KD], F32, tag="t1c", bufs=2)
            gat_pass(st[b]["g0t"], zsrc1_sb[:, b:b + 1], st[b]["nm24"],
                     gce1, t1c)
            st[b]["gce1"], st[b]["t1c"] = gce1, t1c

        def stage_V1(b):
            """t1 layernorm folded into zsrc2 algebraically:
            zsrc2 = r*(wzt.T@t1c - m*colsum(wzt)), so the only PE matmuls
            wait just on the t1c tanh, not on the whole stats chain."""
            t1c = st[b]["t1c"]
            nc.scalar.activation(t1c, t1c, AF.Tanh)
            tsq = sm.tile([128, KD], BF16, tag="tsq")
            nc.vector.tensor_mul(tsq, t1c, t1c)
            tcb = sm.tile([128, KD], BF16, tag="tcb")
            nc.vector.tensor_copy(tcb, t1c)
            pm = psb.tile([128, 512], F32, tag="b1")
            pq = psb.tile([128, 512], F32, tag="b1")
            zraw = psb.tile([128, 512], F32, tag="b1")
            for k in range(KD):
                nc.tensor.matmul(pm[0:1, 0:1], lhsT=onescol_sb,
                                 rhs=tcb[:, k:k + 1],
                                 start=(k == 0), stop=(k == KD - 1))
                nc.tensor.matmul(pq[0:1, 0:1], lhsT=onescol_sb,
                                 rhs=tsq[:, k:k + 1],
                                 start=(k == 0), stop=(k == KD - 1))
                nc.tensor.matmul(zraw[0:24, 0:1], lhsT=wzt_sb[:, k, :],
                                 rhs=tcb[:, k:k + 1],
                                 start=(k == 0), stop=(k == KD - 1))
            mean = sm.tile([1, 1], F32, tag="tst")
            nc.vector.tensor_scalar(out=mean, in0=pm[0:1, 0:1], scalar1=1.0 / D,
                                    scalar2=None, op0=OP.mult)
            msq = sm.tile([1, 1], F32, tag="tst")
            nc.vector.tensor_mul(msq, mean, mean)
            var = sm.tile([1, 1], F32, tag="tst")
            nc.vector.scalar_tensor_tensor(out=var, in0=pq[0:1, 0:1],
                                           scalar=1.0 / D, in1=msq,
                                           op0=OP.mult, op1=OP.subtract)
            lnv = sm.tile([1, 1], F32, tag="tst")
            nc.scalar.activation(lnv, var, AF.Ln, bias=eps_sb[1e-5][:1, :])
            rs = sm.tile([1, 1], F32, tag="tst")
            nc.scalar.activation(rs, lnv, AF.Exp, scale=-0.5)
            # broadcast mean/rs to 24 partitions (free-size-1 matmuls)
            p24 = psb.tile([128, 512], F32, tag="b1")
            nc.tensor.matmul(p24[0:24, 0:1], lhsT=onesrow_sb[:, 0:24],
                             rhs=mean, start=True, stop=False)
            nc.tensor.matmul(p24[0:24, 1:2], lhsT=onesrow_sb[:, 0:24],
                             rhs=rs, start=False, stop=True)
            mr24 = sm.tile([24, 2], F32, tag="mr24", bufs=2)
            nc.vector.tensor_copy(mr24, p24[0:24, 0:2])
            mc24 = sm.tile([24, 1], F32, tag="mr24", bufs=2)
            nc.vector.tensor_mul(mc24, mr24[:, 0:1], czt_sb)
            zsrc2 = sm.tile([24, 1], F32, tag="zsrc2", bufs=2)
            nc.vector.scalar_tensor_tensor(out=zsrc2, in0=zraw[0:24, 0:1],
                                           scalar=mc24, in1=mr24[:, 1:2],
                                           op0=OP.subtract, op1=OP.mult)
            st[b]["zsrc2"] = zsrc2

        def stage_S1(b):
            mb1 = sm.tile([128, S], BF16, tag="lnmb", bufs=2)
            rb1 = sm.tile([128, S], BF16, tag="lnrb", bufs=2)
            feat_ln(st[b]["gce1"], 1e-5, mb1, rb1)
            st[b]["mb1"], st[b]["rb1"] = mb1, rb1

        def stage_A1(b):
            g1ln = chunk_tiles("actB", 12)
            ln_apply(st[b]["gce1"], st[b]["mb1"], st[b]["rb1"], g1ln)
            st[b]["g1ln"] = g1ln

        def stage_P2(b):
            gce2 = chunk_tiles("actA", 12)
            gat_pass(st[b]["g1ln"], st[b]["zsrc2"], st[b]["nm24"],
                     gce2, t2_f[:, :, b])
            st[b]["gce2"] = gce2

        def stage_V2(b):
            # t2 tanh + layernorm for the out_tp tail; pure DVE/ACT latency,
            # emitted while the FFN keeps PE busy.
            nc.scalar.activation(t2_f[:, :, b], t2_f[:, :, b], AF.Tanh)
            nc.vector.tensor_copy(t2_b[:, :, b], t2_f[:, :, b])
            t2ln = vec_ln(t2_f[:, :, b], 1e-6)
            nc.vector.tensor_copy(t2ln_sb[:, :, b], t2ln)

        def stage_S2(b):
            mb2 = sm.tile([128, S], BF16, tag="lnmb", bufs=2)
            rb2 = sm.tile([128, S], BF16, tag="lnrb", bufs=2)
            feat_ln(st[b]["gce2"], 1e-6, mb2, rb2)
            st[b]["mb2"], st[b]["rb2"] = mb2, rb2

        def stage_A2(b):
            g2ln = chunk_tiles("actB", 12)
            ln_apply(st[b]["gce2"], st[b]["mb2"], st[b]["rb2"], g2ln)
            st[b]["g2ln"] = g2ln

        def stage_F(b):
            gce2, g2ln = st[b]["gce2"], st[b]["g2ln"]
            for qq in range(4):
                qsl = slice(qq * 256, (qq + 1) * 256)
                _namectr[0] += 1
                intT = [acts.tile([128, 512], BF16, tag="intT", bufs=18,
                                  name=f"intT_{_namectr[0]}_{f}")
                        for f in range(KF // 2)]
                for fp in range(KF // 2):
                    ip = psb.tile([128, 512], F32, tag="b1")
                    for ff in range(2):
                        f = 2 * fp + ff
                        csl = slice(ff * 256, (ff + 1) * 256)
                        for k in range(KD):
                            nc.tensor.matmul(ip[:, csl],
                                             lhsT=w1_sb[:, k, f * 128:(f + 1) * 128],
                                             rhs=g2ln[k][:, qsl],
                                             start=(k == 0), stop=(k == KD - 1))
                    if fp % 2 == 0:
                        nc.scalar.activation(intT[fp], ip, AF.Relu)
                    else:
                        nc.vector.tensor_scalar_max(intT[fp], ip, 0.0)
                for j4 in range(2):
                    sj = qq * 2 + j4
                    jsl = slice(j4 * 128, (j4 + 1) * 128)
                    for half in range(2):
                        osl = slice(half * 384, (half + 1) * 384)
                        op_ = psb.tile([128, 512], F32, tag="b1")
                        for f in range(KF):
                            lsl = slice((f % 2) * 256 + j4 * 128,
                                        (f % 2) * 256 + (j4 + 1) * 128)
                            nc.tensor.matmul(op_[:, 0:384],
                                             lhsT=intT[f // 2][:, lsl],
                                             rhs=w2_sb[:, f, osl],
                                             start=(f == 0), stop=False)
                        for jj in range(3):
                            j = half * 3 + jj
                            nc.tensor.matmul(op_[:, jj * 128:(jj + 1) * 128],
                                             lhsT=gce2[j][:, sj * 128:(sj + 1) * 128],
                                             rhs=i128b_sb, start=False,
                                             stop=(jj == 2))
                        osb = outp.tile([128, 384], F32, tag="osb")
                        nc.scalar.activation(osb, op_[:, 0:384], AF.Copy)
                        if STORES_ON_POOL:
                            nc.gpsimd.dma_start(
                                out=outgl_d[b, sj * 128:(sj + 1) * 128, osl],
                                in_=osb)
                        else:
                            nc.sync.dma_start(
                                out=outgl_d[b, sj * 128:(sj + 1) * 128, osl],
                                in_=osb)

        stage_T(0)
        # big FFN weights load behind the first transposes; wc per-chunk so
        # the first fcg matmuls start as soon as chunk 0 lands
        for k in range(KD):
            nc.sync.dma_start(out=wc_sb[k],
                              in_=wc_d[k * 128:(k + 1) * 128, :])
        nc.sync.dma_start(out=w1_sb, in_=w1_d.rearrange("(k p) d -> p k d", p=128))
        nc.sync.dma_start(out=w2_sb, in_=w2_d.rearrange("(k p) d -> p k d", p=128))
        nc.sync.dma_start(out=i128b_sb, in_=i128b_d)
        stage_P1(0)
        stage_S1(0)
        stage_V1(0)
        for b in range(nb):
            stage_A1(b)
            stage_P2(b)
            stage_S2(b)
            if b + 1 < nb:
                stage_T(b + 1)
                stage_P1(b + 1)
                stage_S1(b + 1)
            stage_A2(b)
            stage_F(b)
            if b + 1 < nb:
                stage_V1(b + 1)
            stage_V2(b)

        # ================= out_tp rows (all batches) =================
        itp = psb.tile([128, 512], F32, tag="b1")
        for f in range(KF):
            for k in range(KD):
                nc.tensor.matmul(itp[:, f * nb:(f + 1) * nb],
                                 lhsT=w1_sb[:, k, f * 128:(f + 1) * 128],
                                 rhs=t2ln_sb[:, k, :],
                                 start=(f == 0 and k == 0),
                                 stop=(f == KF - 1 and k == KD - 1))
        itp_sb = sm.tile([128, KF * nb], BF16, tag="itp")
        nc.scalar.activation(itp_sb, itp[:, 0:KF * nb], AF.Relu)
        for half in range(2):
            osl = slice(half * 384, (half + 1) * 384)
            otp = psb.tile([128, 512], F32, tag="b1")
            for f in range(KF):
                nc.tensor.matmul(otp[0:nb, 0:384],
                                 lhsT=itp_sb[:, f * nb:(f + 1) * nb],
                                 rhs=w2_sb[:, f, osl],
                                 start=(f == 0), stop=False)
            for jj in range(3):
                j = half * 3 + jj
                nc.tensor.matmul(otp[0:nb, jj * 128:(jj + 1) * 128],
                                 lhsT=t2_b[:, j, :],
                                 rhs=i128b_sb, start=False, stop=(jj == 2))
            otp_sb = outp.tile([nb, 384], F32, tag="otp")
            nc.scalar.activation(otp_sb, otp[0:nb, 0:384], AF.Copy)
            nc.sync.dma_start(out=outtp_d[:, osl], in_=otp_sb)

    _split_multi_waits(nc, dummy_sem)
    return nc


# ---------------------------------------------------------------------------
# host side
# ---------------------------------------------------------------------------

def host_prep(inputs):
    """Fold weights; build constants. Returns dict of shared arrays."""
    Wt = np.asarray(inputs["Wt"], np.float32)
    Wg = np.asarray(inputs["Wg"], np.float32)
    Wc = np.asarray(inputs["Wc"], np.float32)
    Wa = np.asarray(inputs["Wa"], np.float32)
    Wa1 = np.asarray(inputs["Wa1"], np.float32)

    wc = np.ascontiguousarray(np.transpose(Wc, (1, 0, 2)).reshape(D, D))
    wz = np.concatenate([np.einsum("hid,hd->ih", Wg, Wa[:, DH:]),
                         np.einsum("hid,hd->ih", Wg, Wa1[:, DH:])], axis=1)
    wzt = np.concatenate([np.einsum("hid,hd->ih", Wt, Wa[:, :DH]),
                          np.einsum("hid,hd->ih", Wt, Wa1[:, :DH])], axis=1)

    hmap = (np.arange(D) // DH)  # feature -> head
    ea = np.zeros((24, D), np.float32)
    ea[hmap, np.arange(D)] = 1.0          # rows 0..11 select attn-a
    ea1 = np.zeros((24, D), np.float32)
    ea1[12 + hmap, np.arange(D)] = 1.0    # rows 12..23 select attn-a1

    return {
        "wc": wc.astype(BF), "wz": wz.astype(BF), "wzt": wzt.astype(BF),
        "w1": np.asarray(inputs["pw_w1"], np.float32).astype(BF),
        "w2": np.asarray(inputs["pw_w2"], np.float32).astype(BF),
        "ea": ea.astype(BF), "ea1": ea1.astype(BF),
        "i128b": np.eye(128, dtype=np.float32).astype(BF),
        "ones128": np.ones((128, 128), np.float32).astype(BF),
        "onescol": np.ones((128, 1), np.float32).astype(BF),
        "onesrow": np.ones((1, 128), np.float32),
        "ones24": np.ones((1, 24), np.float32).astype(BF),
        "czt": np.ascontiguousarray(wzt.sum(axis=0).reshape(24, 1)),
    }


def core_inputs(inputs, shared, c, nb=NB):
    """Per-core in_map (core c takes batches c*nb .. c*nb+nb)."""
    sl = slice(c * nb, c * nb + nb)
    gce = np.ascontiguousarray(np.asarray(
        inputs["global_context_embed"], np.float32)[sl]).astype(BF)
    mask = np.asarray(inputs["mask"])[sl]
    negmask = np.where(mask, np.float32(NEG), np.float32(0.0)).astype(BF)
    topict = np.ascontiguousarray(
        np.asarray(inputs["topic_embed"], np.float32).T[:, sl]).astype(BF)
    m = dict(shared)
    m.update({"gce": gce, "negmask": negmask, "topict": topict})
    return m


_prog_cache = {}


def _get_program(nb=NB):
    if nb not in _prog_cache:
        _prog_cache[nb] = build_program(nb)
    return _prog_cache[nb]


def kernel(**inputs):
    nc = _get_program()
    shared = host_prep(inputs)
    in_maps = [core_inputs(inputs, shared, c) for c in range(NCORES)]
    res = run_bass_kernel_spmd(nc, in_maps, list(range(NCORES)))
    outgl = np.concatenate([res.results[c]["outgl"] for c in range(NCORES)], axis=0)
    tprow = np.concatenate([res.results[c]["outtp"] for c in range(NCORES)], axis=0)
    out_tp = np.broadcast_to(tprow[:, None, :], (B, S, D))
    return np.ascontiguousarray(outgl), np.ascontiguousarray(out_tp)
